# revision 29
# baseline (speedup 1.0000x reference)
"""AUGRU cell (attention-scaled GRU update) on 8 Trainium2 NeuronCores.

Data-parallel: batch B=65536 sharded 8 ways (8192 rows/core); gate weights
replicated.  Per core (gate-major layout, batch on the free axis):

  gates_x = x @ W_x.T + b_x
  gates_h = h @ W_h.T + b_h
  u = sigmoid(U); r = sigmoid(R); t = tanh(Cx + r*Ch)
  h_new = h + att*u*(t - h)

v12 design -- 7 matmuls/group, ACT-bias sigmoids, R-first critical path:
  - biases enter via the ACT bias operand (per-partition [P,1]) -> no K=1
    bias-prefill matmuls.  sigU/sigR are one [P,1024] ACT op per pair.
  - R gate is computed FIRST (its sigmoid feeds m -> identity-matmul ->
    tanh, the longest chain); U follows; Ch before Cx so m is never gated
    behind the pcx WAR.
  - PSUM: pu pair [P,2,512] bufs=1 (2 banks) + pr pair bufs=1 (2) +
    pcx group bufs=2 (2) + pch group bufs=2 (2) = 8 banks; split U/R pools
    release matmul WARs right after each sigmoid instead of after both.
  - tanh per group emitted in the same stage as m/id so the Cx bank WAR
    releases early; identity matmul merges m = (Ch+bCh)*r into the open
    Cx bank (216ns on PE vs ~0.75us on DVE).
  - head: first x/h slices issued from the scalar queue in parallel with
    the sync queue's weight/attb DMAs; bulk of x/h as single 1.5MiB DMAs.
  - epilogue per pair (1024 cols): ua=att*u (DVE), d=t-h (GPSIMD),
    q=ua*d (DVE), ho=h+q (DVE), one output DMA; last pair split per group.
"""

import sys

sys.path.insert(0, "/opt/trn_rl_repo")

from contextlib import ExitStack

import numpy as np
import ml_dtypes

import concourse.bass as bass
import concourse.tile as tile
from concourse import bacc, mybir
from concourse.bass_utils import run_bass_kernel_spmd

F32 = mybir.dt.float32
BF16 = mybir.dt.bfloat16
AF = mybir.ActivationFunctionType
OP = mybir.AluOpType
BFNP = ml_dtypes.bfloat16

B = 65536
NCORES = 8
BL = B // NCORES  # 8192 rows per core
I = 128
H = 128
P = 128
ROWS = 512  # batch rows per group (one fp32 PSUM bank per gate)
NGROUPS = BL // ROWS  # 16
NP = NGROUPS // 2  # 8 pairs
PR = 2 * ROWS  # pair width 1024


def build_program():
    nc = bacc.Bacc("TRN2", target_bir_lowering=False, debug=False)

    xT_d = nc.dram_tensor("xT", [I, BL], BF16, kind="ExternalInput").ap()
    hT_d = nc.dram_tensor("hT", [H, BL], BF16, kind="ExternalInput").ap()
    ab_d = nc.dram_tensor("attb", [P, BL], BF16, kind="ExternalInput").ap()
    wx_d = nc.dram_tensor("wxT", [I, 3, P], BF16, kind="ExternalInput").ap()
    wh_d = nc.dram_tensor("whT", [H, 3, P], BF16, kind="ExternalInput").ap()
    bc_d = nc.dram_tensor("bcol", [P, 4], F32, kind="ExternalInput").ap()
    id_d = nc.dram_tensor("ident", [P, P], BF16, kind="ExternalInput").ap()
    o_d = nc.dram_tensor("h_newT", [H, BL], BF16, kind="ExternalOutput").ap()

    with tile.TileContext(nc) as tc, ExitStack() as ctx:
        consts = ctx.enter_context(tc.tile_pool(name="consts", bufs=1))
        io = ctx.enter_context(tc.tile_pool(name="io", bufs=1))
        gp = ctx.enter_context(tc.tile_pool(name="gp", bufs=3))
        ep = ctx.enter_context(tc.tile_pool(name="ep", bufs=4))
        pu = ctx.enter_context(tc.tile_pool(name="pu", bufs=1, space="PSUM"))
        pr = ctx.enter_context(tc.tile_pool(name="pr", bufs=1, space="PSUM"))
        pcx = ctx.enter_context(tc.tile_pool(name="pcx", bufs=2, space="PSUM"))
        pch = ctx.enter_context(tc.tile_pool(name="pch", bufs=1, space="PSUM"))

        # ---------------- one-time setup ----------------
        # whole-core input/attb tiles; DMAs fill column ranges
        xs = io.tile([P, BL], BF16, tag="xs")
        hs = io.tile([P, BL], BF16, tag="hs")
        ab = io.tile([P, BL], BF16, tag="ab")
        wT = consts.tile([P, 6, P], BF16, tag="wT")  # [xu, xr, xc, hu, hr, hc]
        bcol = consts.tile([P, 4], F32, tag="bcol")  # [bU, bR, bCx, bCh]
        ident = consts.tile([P, P], BF16, tag="ident")

        # pre-trigger the ACT_TABLE_LOAD with a dummy activation on a
        # memset tile (no DMA dependency): the sigmoid/tanh table is resident
        # before the first pair's matmuls retire.
        dmy = consts.tile([1, 2], F32, name="dmy")
        dmy2 = consts.tile([1, 2], F32, name="dmy2")
        nc.vector.memset(dmy, 0.0)
        nc.scalar.activation(dmy2, dmy, AF.Sigmoid, bias=0.0)
        # scalar queue carries no other work before the first real sigmoid.
        # sync HWDGE ring is FIFO: first-pair slices + weights first, then
        # pair-granular x/h through pair 3 in consumption order.
        # bcol first: it gates the ACT_TABLE_LOAD that gates the first sigmoid
        nc.sync.dma_start(bcol, bc_d)
        nc.sync.dma_start(wT[:, 0:3, :], wx_d)
        nc.sync.dma_start(wT[:, 3:6, :], wh_d)
        nc.sync.dma_start(xs[:, 0:ROWS], xT_d[:, 0:ROWS])
        nc.sync.dma_start(hs[:, 0:ROWS], hT_d[:, 0:ROWS])
        nc.sync.dma_start(xs[:, ROWS:PR], xT_d[:, ROWS:PR])
        nc.sync.dma_start(hs[:, ROWS:PR], hT_d[:, ROWS:PR])
        nc.sync.dma_start(ident, id_d)
        for p in range(1, 4):
            lo, hi = p * PR, (p + 1) * PR
            nc.sync.dma_start(xs[:, lo:hi], xT_d[:, lo:hi])
            nc.sync.dma_start(hs[:, lo:hi], hT_d[:, lo:hi])
        nc.sync.dma_start(ab[:, 0 : 2 * PR], ab_d[:, 0 : 2 * PR])
        nc.sync.dma_start(xs[:, 4 * PR : 6 * PR], xT_d[:, 4 * PR : 6 * PR])
        nc.sync.dma_start(hs[:, 4 * PR : 6 * PR], hT_d[:, 4 * PR : 6 * PR])
        nc.sync.dma_start(ab[:, 2 * PR : 4 * PR], ab_d[:, 2 * PR : 4 * PR])
        nc.sync.dma_start(xs[:, 6 * PR :], xT_d[:, 6 * PR :])
        nc.sync.dma_start(hs[:, 6 * PR :], hT_d[:, 6 * PR :])
        nc.sync.dma_start(ab[:, 4 * PR :], ab_d[:, 4 * PR :])

        # work items: first/last two groups run as 512-wide halves to shorten
        # pipeline ramp and drain; the middle 12 groups as 1024-wide pairs
        ITEMS = [[0], [1]] + [[g, g + 1] for g in range(2, 16, 2)]
        NI = len(ITEMS)  # 10
        stB = [None] * NI
        uts = [None] * NI  # (u_ap [P, n*ROWS], t_ap) flat views per item

        def stage_b(i):
            gs = ITEMS[i]
            xg = [xs[:, g * ROWS : (g + 1) * ROWS] for g in gs]
            hg = [hs[:, g * ROWS : (g + 1) * ROWS] for g in gs]
            n = len(gs)
            u_ps = pu.tile([P, 2, ROWS], F32, tag="u_ps")
            r_ps = pr.tile([P, 2, ROWS], F32, tag="r_ps")
            # R first: its sigmoid heads the m -> id -> tanh chain
            for j in range(n):
                nc.tensor.matmul(r_ps[:, j, :], lhsT=wT[:, 1, :], rhs=xg[j], start=True, stop=False)
            for j in range(n):
                nc.tensor.matmul(r_ps[:, j, :], lhsT=wT[:, 4, :], rhs=hg[j], start=False, stop=True)
            for j in range(n):
                nc.tensor.matmul(u_ps[:, j, :], lhsT=wT[:, 0, :], rhs=xg[j], start=True, stop=False)
            for j in range(n):
                nc.tensor.matmul(u_ps[:, j, :], lhsT=wT[:, 3, :], rhs=hg[j], start=False, stop=True)
            ch = pch.tile([P, 2, ROWS], F32, tag="ch")
            cxs = []
            for j in range(n):
                nc.tensor.matmul(ch[:, j, :], lhsT=wT[:, 5, :], rhs=hg[j], start=True, stop=True)
            for j in range(n):
                cx = pcx.tile([P, ROWS], F32, tag="cx")
                nc.tensor.matmul(cx, lhsT=wT[:, 2, :], rhs=xg[j], start=True, stop=False)  # stays open
                cxs.append(cx)
            stB[i] = (u_ps, r_ps, cxs, ch)

        u01 = [None]  # shared pair tiles for the two head halves
        t01 = [None]
        uq = {}  # quad tiles for middle pairs, keyed by quad index
        tq = {}

        def stage_c(i):
            u_ps, r_ps, cxs, ch = stB[i]
            gs = ITEMS[i]
            n = len(gs)
            if i <= 1:
                if i == 0:
                    u01[0] = gp.tile([P, 2, ROWS], BF16, tag="u", name="uh01")
                    t01[0] = gp.tile([P, 2, ROWS], BF16, tag="t", name="th01")
                u, t = u01[0][:, i : i + 1, :], t01[0][:, i : i + 1, :]
            elif i == NI - 1:
                # final pair keeps its own tiles for a group-split drain
                u = gp.tile([P, 2, ROWS], BF16, tag="u", name="ulast")
                t = gp.tile([P, 2, ROWS], BF16, tag="t", name="tlast")
                uts[i] = (u.rearrange("p a b -> p (a b)"), t.rearrange("p a b -> p (a b)"))
            else:
                qd, ph = (i - 2) // 2, (i - 2) % 2
                if ph == 0:
                    uq[qd] = gp.tile([P, 2, 2, ROWS], BF16, tag="u", name="uquad")
                    tq[qd] = gp.tile([P, 2, 2, ROWS], BF16, tag="t", name="tquad")
                u, t = uq[qd][:, ph, :, :], tq[qd][:, ph, :, :]
            r = gp.tile([P, 2, ROWS], BF16, tag="r")
            m = gp.tile([P, 2, ROWS], BF16, tag="m")
            nc.scalar.activation(r[:, 0:n, :], r_ps[:, 0:n, :], AF.Sigmoid, bias=bcol[:, 1:2])
            # m per group: shortens the sigR -> m -> id -> tanh_g0 chain so
            # tanh_g0 is ready right as sigU retires (zero ACT bubble)
            for j in range(n):
                nc.vector.scalar_tensor_tensor(
                    m[:, j, :], in0=ch[:, j, :], scalar=bcol[:, 3:4], in1=r[:, j, :],
                    op0=OP.add, op1=OP.mult,
                )
                nc.tensor.matmul(cxs[j], lhsT=ident, rhs=m[:, j, :], start=False, stop=True)
            nc.scalar.activation(u, u_ps[:, 0:n, :], AF.Sigmoid, bias=bcol[:, 0:1])
            for j in range(n):
                nc.scalar.activation(t[:, j, :], cxs[j], AF.Tanh, bias=bcol[:, 2:3])

        def epilogue(uf, tf, base, width):
            hsl = hs[:, base : base + width]
            ua = ep.tile([P, width], BF16, tag="ua", name="ua")
            d = ep.tile([P, width], BF16, tag="d", name="d")
            q = ep.tile([P, width], BF16, tag="q", name="q")
            ho = ep.tile([P, width], BF16, tag="ho", name="ho")
            nc.vector.tensor_tensor(ua, uf, ab[:, base : base + width], OP.mult)
            nc.vector.tensor_tensor(d, tf, hsl, OP.subtract)
            nc.vector.tensor_tensor(q, d, ua, OP.mult)
            nc.vector.tensor_tensor(ho, q, hsl, OP.add)
            nc.sync.dma_start(o_d[:, base : base + width], ho)

        QR = 4 * ROWS  # quad width 2048

        for k in range(NI + 2):
            if k < NI:
                stage_b(k)
            if 1 <= k < NI + 1:
                stage_c(k - 1)
            if k == 3:
                # head halves (groups 0,1) as one 1024-wide epilogue
                epilogue(u01[0].rearrange("p a b -> p (a b)"),
                         t01[0].rearrange("p a b -> p (a b)"), 0, PR)
            if k in (5, 7, 9):
                # quads over item pairs (I2,I3),(I4,I5),(I6,I7)
                qd = (k - 5) // 2
                epilogue(uq[qd].rearrange("p a b c -> p (a b c)"),
                         tq[qd].rearrange("p a b c -> p (a b c)"), PR + qd * QR, QR)
            if k == NI + 1:
                # final pair: per-group chains, first half drains early
                i = NI - 1
                base = ITEMS[i][0] * ROWS
                uf, tf = uts[i]
                for g in range(2):
                    epilogue(uf[:, g * ROWS : (g + 1) * ROWS],
                             tf[:, g * ROWS : (g + 1) * ROWS], base + g * ROWS, ROWS)

    nc.compile()
    return nc


_NC_CACHE = []


def _get_nc():
    if not _NC_CACHE:
        _NC_CACHE.append(build_program())
    return _NC_CACHE[0]


def make_in_maps(x, h_prev, att_score, W_x, b_x, W_h, b_h):
    """Shard + stage inputs for the 8 cores (bf16 wire format)."""
    x = np.asarray(x, dtype=np.float32)
    h_prev = np.asarray(h_prev, dtype=np.float32)
    att = np.asarray(att_score, dtype=np.float32)
    W_x = np.asarray(W_x, dtype=np.float32)
    W_h = np.asarray(W_h, dtype=np.float32)
    b_x = np.asarray(b_x, dtype=np.float32)
    b_h = np.asarray(b_h, dtype=np.float32)

    wxT = np.ascontiguousarray(W_x.T.reshape(I, 3, P).astype(BFNP))
    whT = np.ascontiguousarray(W_h.T.reshape(H, 3, P).astype(BFNP))
    bsum = b_x + b_h  # valid for U and R blocks
    bcol = np.stack(
        [bsum[0:P], bsum[P : 2 * P], b_x[2 * P : 3 * P], b_h[2 * P : 3 * P]], axis=1
    ).astype(np.float32)
    ident = np.eye(P, dtype=BFNP)

    in_maps = []
    for c in range(NCORES):
        s = slice(c * BL, (c + 1) * BL)
        attb = np.broadcast_to(att[s].astype(BFNP), (P, BL))
        in_maps.append(
            {
                "xT": np.ascontiguousarray(x[s].T.astype(BFNP)),
                "hT": np.ascontiguousarray(h_prev[s].T.astype(BFNP)),
                "attb": np.ascontiguousarray(attb),
                "wxT": wxT,
                "whT": whT,
                "bcol": bcol,
                "ident": ident,
            }
        )
    return in_maps


def kernel(x, h_prev, att_score, W_x, b_x, W_h, b_h, **_unused):
    nc = _get_nc()
    in_maps = make_in_maps(x, h_prev, att_score, W_x, b_x, W_h, b_h)
    res = run_bass_kernel_spmd(nc, in_maps, list(range(NCORES)))
    out = np.concatenate(
        [
            np.asarray(res.results[c]["h_newT"]).astype(np.float32).T
            for c in range(NCORES)
        ],
        axis=0,
    )
    return np.ascontiguousarray(out)


# revision 30
# speedup vs baseline: 1.0114x; 1.0114x over previous
"""AUGRU cell (attention-scaled GRU update) on 8 Trainium2 NeuronCores.

Data-parallel: batch B=65536 sharded 8 ways (8192 rows/core); gate weights
replicated.  Per core (gate-major layout, batch on the free axis):

  gates_x = x @ W_x.T + b_x
  gates_h = h @ W_h.T + b_h
  u = sigmoid(U); r = sigmoid(R); t = tanh(Cx + r*Ch)
  h_new = h + att*u*(t - h)

v12 design -- 7 matmuls/group, ACT-bias sigmoids, R-first critical path:
  - biases enter via the ACT bias operand (per-partition [P,1]) -> no K=1
    bias-prefill matmuls.  sigU/sigR are one [P,1024] ACT op per pair.
  - R gate is computed FIRST (its sigmoid feeds m -> identity-matmul ->
    tanh, the longest chain); U follows; Ch before Cx so m is never gated
    behind the pcx WAR.
  - PSUM: pu pair [P,2,512] bufs=1 (2 banks) + pr pair bufs=1 (2) +
    pcx group bufs=2 (2) + pch group bufs=2 (2) = 8 banks; split U/R pools
    release matmul WARs right after each sigmoid instead of after both.
  - tanh per group emitted in the same stage as m/id so the Cx bank WAR
    releases early; identity matmul merges m = (Ch+bCh)*r into the open
    Cx bank (216ns on PE vs ~0.75us on DVE).
  - head: first x/h slices issued from the scalar queue in parallel with
    the sync queue's weight/attb DMAs; bulk of x/h as single 1.5MiB DMAs.
  - epilogue per pair (1024 cols): ua=att*u (DVE), d=t-h (GPSIMD),
    q=ua*d (DVE), ho=h+q (DVE), one output DMA; last pair split per group.
"""

import sys

sys.path.insert(0, "/opt/trn_rl_repo")

from contextlib import ExitStack

import numpy as np
import ml_dtypes

import concourse.bass as bass
import concourse.tile as tile
from concourse import bacc, mybir
from concourse.bass_utils import run_bass_kernel_spmd

F32 = mybir.dt.float32
BF16 = mybir.dt.bfloat16
AF = mybir.ActivationFunctionType
OP = mybir.AluOpType
BFNP = ml_dtypes.bfloat16

B = 65536
NCORES = 8
BL = B // NCORES  # 8192 rows per core
I = 128
H = 128
P = 128
ROWS = 512  # batch rows per group (one fp32 PSUM bank per gate)
NGROUPS = BL // ROWS  # 16
NP = NGROUPS // 2  # 8 pairs
PR = 2 * ROWS  # pair width 1024


def build_program():
    nc = bacc.Bacc("TRN2", target_bir_lowering=False, debug=False)

    xT_d = nc.dram_tensor("xT", [I, BL], BF16, kind="ExternalInput").ap()
    hT_d = nc.dram_tensor("hT", [H, BL], BF16, kind="ExternalInput").ap()
    ab_d = nc.dram_tensor("attb", [P, BL], BF16, kind="ExternalInput").ap()
    wx_d = nc.dram_tensor("wxT", [I, 3, P], BF16, kind="ExternalInput").ap()
    wh_d = nc.dram_tensor("whT", [H, 3, P], BF16, kind="ExternalInput").ap()
    bc_d = nc.dram_tensor("bcol", [P, 4], F32, kind="ExternalInput").ap()
    id_d = nc.dram_tensor("ident", [P, P], BF16, kind="ExternalInput").ap()
    o_d = nc.dram_tensor("h_newT", [H, BL], BF16, kind="ExternalOutput").ap()

    with tile.TileContext(nc) as tc, ExitStack() as ctx:
        consts = ctx.enter_context(tc.tile_pool(name="consts", bufs=1))
        io = ctx.enter_context(tc.tile_pool(name="io", bufs=1))
        gp = ctx.enter_context(tc.tile_pool(name="gp", bufs=2))
        ep = ctx.enter_context(tc.tile_pool(name="ep", bufs=3))
        pu = ctx.enter_context(tc.tile_pool(name="pu", bufs=1, space="PSUM"))
        pr = ctx.enter_context(tc.tile_pool(name="pr", bufs=1, space="PSUM"))
        pcx = ctx.enter_context(tc.tile_pool(name="pcx", bufs=2, space="PSUM"))
        pch = ctx.enter_context(tc.tile_pool(name="pch", bufs=1, space="PSUM"))

        # ---------------- one-time setup ----------------
        # whole-core input/attb tiles; DMAs fill column ranges
        xs = io.tile([P, BL], BF16, tag="xs")
        hs = io.tile([P, BL], BF16, tag="hs")
        ab = io.tile([P, BL], BF16, tag="ab")
        wT = consts.tile([P, 6, P], BF16, tag="wT")  # [xu, xr, xc, hu, hr, hc]
        bcol = consts.tile([P, 4], F32, tag="bcol")  # [bU, bR, bCx, bCh]
        ident = consts.tile([P, P], BF16, tag="ident")

        # pre-trigger the ACT_TABLE_LOAD with a dummy activation on a
        # memset tile (no DMA dependency): the sigmoid/tanh table is resident
        # before the first pair's matmuls retire.
        dmy = consts.tile([1, 2], F32, name="dmy")
        dmy2 = consts.tile([1, 2], F32, name="dmy2")
        nc.vector.memset(dmy, 0.0)
        nc.scalar.activation(dmy2, dmy, AF.Sigmoid, bias=0.0)
        # scalar queue carries no other work before the first real sigmoid.
        # sync HWDGE ring is FIFO: first-pair slices + weights first, then
        # pair-granular x/h through pair 3 in consumption order.
        # bcol first: it gates the ACT_TABLE_LOAD that gates the first sigmoid
        nc.sync.dma_start(bcol, bc_d)
        nc.sync.dma_start(wT[:, 0:3, :], wx_d)
        nc.sync.dma_start(wT[:, 3:6, :], wh_d)
        nc.sync.dma_start(xs[:, 0:ROWS], xT_d[:, 0:ROWS])
        nc.sync.dma_start(hs[:, 0:ROWS], hT_d[:, 0:ROWS])
        nc.sync.dma_start(xs[:, ROWS:PR], xT_d[:, ROWS:PR])
        nc.sync.dma_start(hs[:, ROWS:PR], hT_d[:, ROWS:PR])
        nc.sync.dma_start(ident, id_d)
        for p in range(1, 4):
            lo, hi = p * PR, (p + 1) * PR
            nc.sync.dma_start(xs[:, lo:hi], xT_d[:, lo:hi])
            nc.sync.dma_start(hs[:, lo:hi], hT_d[:, lo:hi])
        nc.sync.dma_start(ab[:, 0 : 2 * PR], ab_d[:, 0 : 2 * PR])
        nc.sync.dma_start(xs[:, 4 * PR : 6 * PR], xT_d[:, 4 * PR : 6 * PR])
        nc.sync.dma_start(hs[:, 4 * PR : 6 * PR], hT_d[:, 4 * PR : 6 * PR])
        nc.sync.dma_start(ab[:, 2 * PR : 4 * PR], ab_d[:, 2 * PR : 4 * PR])
        nc.sync.dma_start(xs[:, 6 * PR :], xT_d[:, 6 * PR :])
        nc.sync.dma_start(hs[:, 6 * PR :], hT_d[:, 6 * PR :])
        nc.sync.dma_start(ab[:, 4 * PR :], ab_d[:, 4 * PR :])

        # work items: first/last two groups run as 512-wide halves to shorten
        # pipeline ramp and drain; the middle 12 groups as 1024-wide pairs
        ITEMS = [[0], [1]] + [[g, g + 1] for g in range(2, 16, 2)]
        NI = len(ITEMS)  # 10
        stB = [None] * NI
        uts = [None] * NI  # (u_ap [P, n*ROWS], t_ap) flat views per item

        def stage_b(i):
            gs = ITEMS[i]
            xg = [xs[:, g * ROWS : (g + 1) * ROWS] for g in gs]
            hg = [hs[:, g * ROWS : (g + 1) * ROWS] for g in gs]
            n = len(gs)
            u_ps = pu.tile([P, 2, ROWS], F32, tag="u_ps")
            r_ps = pr.tile([P, 2, ROWS], F32, tag="r_ps")
            # R first: its sigmoid heads the m -> id -> tanh chain
            for j in range(n):
                nc.tensor.matmul(r_ps[:, j, :], lhsT=wT[:, 1, :], rhs=xg[j], start=True, stop=False)
            for j in range(n):
                nc.tensor.matmul(r_ps[:, j, :], lhsT=wT[:, 4, :], rhs=hg[j], start=False, stop=True)
            for j in range(n):
                nc.tensor.matmul(u_ps[:, j, :], lhsT=wT[:, 0, :], rhs=xg[j], start=True, stop=False)
            for j in range(n):
                nc.tensor.matmul(u_ps[:, j, :], lhsT=wT[:, 3, :], rhs=hg[j], start=False, stop=True)
            ch = pch.tile([P, 2, ROWS], F32, tag="ch")
            cxs = []
            for j in range(n):
                nc.tensor.matmul(ch[:, j, :], lhsT=wT[:, 5, :], rhs=hg[j], start=True, stop=True)
            for j in range(n):
                cx = pcx.tile([P, ROWS], F32, tag="cx")
                nc.tensor.matmul(cx, lhsT=wT[:, 2, :], rhs=xg[j], start=True, stop=False)  # stays open
                cxs.append(cx)
            stB[i] = (u_ps, r_ps, cxs, ch)

        u01 = [None]  # shared pair tiles for the two head halves
        t01 = [None]
        uq = {}  # quad tiles for middle pairs, keyed by quad index
        tq = {}

        def stage_c(i):
            u_ps, r_ps, cxs, ch = stB[i]
            gs = ITEMS[i]
            n = len(gs)
            if i <= 1:
                if i == 0:
                    u01[0] = gp.tile([P, 2, ROWS], BF16, tag="u", name="uh01")
                    t01[0] = gp.tile([P, 2, ROWS], BF16, tag="t", name="th01")
                u, t = u01[0][:, i : i + 1, :], t01[0][:, i : i + 1, :]
            elif i == NI - 1:
                # final pair keeps its own tiles for a group-split drain
                u = gp.tile([P, 2, ROWS], BF16, tag="u", name="ulast")
                t = gp.tile([P, 2, ROWS], BF16, tag="t", name="tlast")
                uts[i] = (u.rearrange("p a b -> p (a b)"), t.rearrange("p a b -> p (a b)"))
            else:
                qd, ph = (i - 2) // 2, (i - 2) % 2
                if ph == 0:
                    uq[qd] = gp.tile([P, 2, 2, ROWS], BF16, tag="u", name="uquad")
                    tq[qd] = gp.tile([P, 2, 2, ROWS], BF16, tag="t", name="tquad")
                u, t = uq[qd][:, ph, :, :], tq[qd][:, ph, :, :]
            r = gp.tile([P, 2, ROWS], BF16, tag="r")
            m = gp.tile([P, 2, ROWS], BF16, tag="m")
            nc.scalar.activation(r[:, 0:n, :], r_ps[:, 0:n, :], AF.Sigmoid, bias=bcol[:, 1:2])
            # m per group: shortens the sigR -> m -> id -> tanh_g0 chain so
            # tanh_g0 is ready right as sigU retires (zero ACT bubble)
            for j in range(n):
                nc.vector.scalar_tensor_tensor(
                    m[:, j, :], in0=ch[:, j, :], scalar=bcol[:, 3:4], in1=r[:, j, :],
                    op0=OP.add, op1=OP.mult,
                )
                nc.tensor.matmul(cxs[j], lhsT=ident, rhs=m[:, j, :], start=False, stop=True)
            nc.scalar.activation(u, u_ps[:, 0:n, :], AF.Sigmoid, bias=bcol[:, 0:1])
            for j in range(n):
                nc.scalar.activation(t[:, j, :], cxs[j], AF.Tanh, bias=bcol[:, 2:3])

        def epilogue(uf, tf, base, width):
            hsl = hs[:, base : base + width]
            ua = ep.tile([P, width], BF16, tag="ua", name="ua")
            d = ep.tile([P, width], BF16, tag="d", name="d")
            q = ep.tile([P, width], BF16, tag="q", name="q")
            ho = ep.tile([P, width], BF16, tag="ho", name="ho")
            nc.vector.tensor_tensor(ua, uf, ab[:, base : base + width], OP.mult)
            nc.vector.tensor_tensor(d, tf, hsl, OP.subtract)
            nc.vector.tensor_tensor(q, d, ua, OP.mult)
            nc.vector.tensor_tensor(ho, q, hsl, OP.add)
            nc.sync.dma_start(o_d[:, base : base + width], ho)

        QR = 4 * ROWS  # quad width 2048

        for k in range(NI + 2):
            if k < NI:
                stage_b(k)
            if 1 <= k < NI + 1:
                stage_c(k - 1)
            if k == 3:
                # head halves (groups 0,1) as one 1024-wide epilogue
                epilogue(u01[0].rearrange("p a b -> p (a b)"),
                         t01[0].rearrange("p a b -> p (a b)"), 0, PR)
            if k in (5, 7, 9):
                # quads over item pairs (I2,I3),(I4,I5),(I6,I7)
                qd = (k - 5) // 2
                epilogue(uq[qd].rearrange("p a b c -> p (a b c)"),
                         tq[qd].rearrange("p a b c -> p (a b c)"), PR + qd * QR, QR)
            if k == NI + 1:
                # final pair: per-group chains, first half drains early
                i = NI - 1
                base = ITEMS[i][0] * ROWS
                uf, tf = uts[i]
                for g in range(2):
                    epilogue(uf[:, g * ROWS : (g + 1) * ROWS],
                             tf[:, g * ROWS : (g + 1) * ROWS], base + g * ROWS, ROWS)

    nc.compile()
    return nc


_NC_CACHE = []


def _get_nc():
    if not _NC_CACHE:
        _NC_CACHE.append(build_program())
    return _NC_CACHE[0]


def make_in_maps(x, h_prev, att_score, W_x, b_x, W_h, b_h):
    """Shard + stage inputs for the 8 cores (bf16 wire format)."""
    x = np.asarray(x, dtype=np.float32)
    h_prev = np.asarray(h_prev, dtype=np.float32)
    att = np.asarray(att_score, dtype=np.float32)
    W_x = np.asarray(W_x, dtype=np.float32)
    W_h = np.asarray(W_h, dtype=np.float32)
    b_x = np.asarray(b_x, dtype=np.float32)
    b_h = np.asarray(b_h, dtype=np.float32)

    wxT = np.ascontiguousarray(W_x.T.reshape(I, 3, P).astype(BFNP))
    whT = np.ascontiguousarray(W_h.T.reshape(H, 3, P).astype(BFNP))
    bsum = b_x + b_h  # valid for U and R blocks
    bcol = np.stack(
        [bsum[0:P], bsum[P : 2 * P], b_x[2 * P : 3 * P], b_h[2 * P : 3 * P]], axis=1
    ).astype(np.float32)
    ident = np.eye(P, dtype=BFNP)

    in_maps = []
    for c in range(NCORES):
        s = slice(c * BL, (c + 1) * BL)
        attb = np.broadcast_to(att[s].astype(BFNP), (P, BL))
        in_maps.append(
            {
                "xT": np.ascontiguousarray(x[s].T.astype(BFNP)),
                "hT": np.ascontiguousarray(h_prev[s].T.astype(BFNP)),
                "attb": np.ascontiguousarray(attb),
                "wxT": wxT,
                "whT": whT,
                "bcol": bcol,
                "ident": ident,
            }
        )
    return in_maps


def kernel(x, h_prev, att_score, W_x, b_x, W_h, b_h, **_unused):
    nc = _get_nc()
    in_maps = make_in_maps(x, h_prev, att_score, W_x, b_x, W_h, b_h)
    res = run_bass_kernel_spmd(nc, in_maps, list(range(NCORES)))
    out = np.concatenate(
        [
            np.asarray(res.results[c]["h_newT"]).astype(np.float32).T
            for c in range(NCORES)
        ],
        axis=0,
    )
    return np.ascontiguousarray(out)


# revision 31
# speedup vs baseline: 1.0710x; 1.0590x over previous
"""AUGRU cell (attention-scaled GRU update) on 8 Trainium2 NeuronCores.

Data-parallel: batch B=65536 sharded 8 ways (8192 rows/core); gate weights
replicated.  Per core (gate-major layout, batch on the free axis):

  gates_x = x @ W_x.T + b_x
  gates_h = h @ W_h.T + b_h
  u = sigmoid(U); r = sigmoid(R); t = tanh(Cx + r*Ch)
  h_new = h + att*u*(t - h)

v12 design -- 7 matmuls/group, ACT-bias sigmoids, R-first critical path:
  - biases enter via the ACT bias operand (per-partition [P,1]) -> no K=1
    bias-prefill matmuls.  sigU/sigR are one [P,1024] ACT op per pair.
  - R gate is computed FIRST (its sigmoid feeds m -> identity-matmul ->
    tanh, the longest chain); U follows; Ch before Cx so m is never gated
    behind the pcx WAR.
  - PSUM: pu pair [P,2,512] bufs=1 (2 banks) + pr pair bufs=1 (2) +
    pcx group bufs=2 (2) + pch group bufs=2 (2) = 8 banks; split U/R pools
    release matmul WARs right after each sigmoid instead of after both.
  - tanh per group emitted in the same stage as m/id so the Cx bank WAR
    releases early; identity matmul merges m = (Ch+bCh)*r into the open
    Cx bank (216ns on PE vs ~0.75us on DVE).
  - head: first x/h slices issued from the scalar queue in parallel with
    the sync queue's weight/attb DMAs; bulk of x/h as single 1.5MiB DMAs.
  - epilogue per pair (1024 cols): ua=att*u (DVE), d=t-h (GPSIMD),
    q=ua*d (DVE), ho=h+q (DVE), one output DMA; last pair split per group.
"""

import sys

sys.path.insert(0, "/opt/trn_rl_repo")

from contextlib import ExitStack

import numpy as np
import ml_dtypes

import concourse.bass as bass
import concourse.tile as tile
from concourse import bacc, mybir
from concourse.bass_utils import run_bass_kernel_spmd

F32 = mybir.dt.float32
BF16 = mybir.dt.bfloat16
AF = mybir.ActivationFunctionType
OP = mybir.AluOpType
BFNP = ml_dtypes.bfloat16

B = 65536
NCORES = 8
BL = B // NCORES  # 8192 rows per core
I = 128
H = 128
P = 128
ROWS = 512  # batch rows per group (one fp32 PSUM bank per gate)
NGROUPS = BL // ROWS  # 16
NP = NGROUPS // 2  # 8 pairs
PR = 2 * ROWS  # pair width 1024


def build_program():
    nc = bacc.Bacc("TRN2", target_bir_lowering=False, debug=False)

    xT_d = nc.dram_tensor("xT", [I, BL], BF16, kind="ExternalInput").ap()
    hT_d = nc.dram_tensor("hT", [H, BL], BF16, kind="ExternalInput").ap()
    ab_d = nc.dram_tensor("attb", [P, BL], BF16, kind="ExternalInput").ap()
    wx_d = nc.dram_tensor("wxT", [I, 3, P], BF16, kind="ExternalInput").ap()
    wh_d = nc.dram_tensor("whT", [H, 3, P], BF16, kind="ExternalInput").ap()
    bc_d = nc.dram_tensor("bcol", [P, 4], F32, kind="ExternalInput").ap()
    id_d = nc.dram_tensor("ident", [P, P], BF16, kind="ExternalInput").ap()
    o_d = nc.dram_tensor("h_newT", [H, BL], BF16, kind="ExternalOutput").ap()

    with tile.TileContext(nc) as tc, ExitStack() as ctx:
        consts = ctx.enter_context(tc.tile_pool(name="consts", bufs=1))
        io = ctx.enter_context(tc.tile_pool(name="io", bufs=1))
        gp = ctx.enter_context(tc.tile_pool(name="gp", bufs=2))
        ep = ctx.enter_context(tc.tile_pool(name="ep", bufs=3))
        pu = ctx.enter_context(tc.tile_pool(name="pu", bufs=1, space="PSUM"))
        pr = ctx.enter_context(tc.tile_pool(name="pr", bufs=1, space="PSUM"))
        pcx = ctx.enter_context(tc.tile_pool(name="pcx", bufs=2, space="PSUM"))
        pch = ctx.enter_context(tc.tile_pool(name="pch", bufs=1, space="PSUM"))

        # ---------------- one-time setup ----------------
        # whole-core input/attb tiles; DMAs fill column ranges
        xs = io.tile([P, BL], BF16, tag="xs")
        hs = io.tile([P, BL], BF16, tag="hs")
        ab = io.tile([P, BL], BF16, tag="ab")
        wT = consts.tile([P, 6, P], BF16, tag="wT")  # [xu, xr, xc, hu, hr, hc]
        bcol = consts.tile([P, 4], F32, tag="bcol")  # [bU, bR, bCx, bCh]
        ident = consts.tile([P, P], BF16, tag="ident")

        # scalar queue carries ZERO DMAs: its first instruction is the
        # implicit ACT_TABLE_LOAD, so the first sigmoid can fire as soon as
        # the first pair's matmuls retire.
        # sync HWDGE ring is FIFO: first-pair slices + weights first, then
        # pair-granular x/h through pair 3 in consumption order.
        # bcol first: it gates the ACT_TABLE_LOAD that gates the first sigmoid
        nc.sync.dma_start(bcol, bc_d)
        nc.sync.dma_start(wT[:, 0:3, :], wx_d)
        nc.sync.dma_start(wT[:, 3:6, :], wh_d)
        nc.sync.dma_start(xs[:, 0:ROWS], xT_d[:, 0:ROWS])
        nc.sync.dma_start(hs[:, 0:ROWS], hT_d[:, 0:ROWS])
        nc.sync.dma_start(xs[:, ROWS:PR], xT_d[:, ROWS:PR])
        nc.sync.dma_start(hs[:, ROWS:PR], hT_d[:, ROWS:PR])
        nc.sync.dma_start(ident, id_d)
        for p in range(1, 4):
            lo, hi = p * PR, (p + 1) * PR
            nc.sync.dma_start(xs[:, lo:hi], xT_d[:, lo:hi])
            nc.sync.dma_start(hs[:, lo:hi], hT_d[:, lo:hi])
        nc.sync.dma_start(ab[:, 0 : 2 * PR], ab_d[:, 0 : 2 * PR])
        nc.sync.dma_start(xs[:, 4 * PR : 6 * PR], xT_d[:, 4 * PR : 6 * PR])
        nc.sync.dma_start(hs[:, 4 * PR : 6 * PR], hT_d[:, 4 * PR : 6 * PR])
        nc.sync.dma_start(ab[:, 2 * PR : 4 * PR], ab_d[:, 2 * PR : 4 * PR])
        nc.sync.dma_start(xs[:, 6 * PR :], xT_d[:, 6 * PR :])
        nc.sync.dma_start(hs[:, 6 * PR :], hT_d[:, 6 * PR :])
        nc.sync.dma_start(ab[:, 4 * PR :], ab_d[:, 4 * PR :])

        # work items: first/last two groups run as 512-wide halves to shorten
        # pipeline ramp and drain; the middle 12 groups as 1024-wide pairs
        ITEMS = [[0], [1]] + [[g, g + 1] for g in range(2, 16, 2)]
        NI = len(ITEMS)  # 10
        stB = [None] * NI
        uts = [None] * NI  # (u_ap [P, n*ROWS], t_ap) flat views per item

        def stage_b(i):
            gs = ITEMS[i]
            xg = [xs[:, g * ROWS : (g + 1) * ROWS] for g in gs]
            hg = [hs[:, g * ROWS : (g + 1) * ROWS] for g in gs]
            n = len(gs)
            u_ps = pu.tile([P, 2, ROWS], F32, tag="u_ps")
            r_ps = pr.tile([P, 2, ROWS], F32, tag="r_ps")
            # R first: its sigmoid heads the m -> id -> tanh chain
            for j in range(n):
                nc.tensor.matmul(r_ps[:, j, :], lhsT=wT[:, 1, :], rhs=xg[j], start=True, stop=False)
            for j in range(n):
                nc.tensor.matmul(r_ps[:, j, :], lhsT=wT[:, 4, :], rhs=hg[j], start=False, stop=True)
            for j in range(n):
                nc.tensor.matmul(u_ps[:, j, :], lhsT=wT[:, 0, :], rhs=xg[j], start=True, stop=False)
            for j in range(n):
                nc.tensor.matmul(u_ps[:, j, :], lhsT=wT[:, 3, :], rhs=hg[j], start=False, stop=True)
            ch = pch.tile([P, 2, ROWS], F32, tag="ch")
            cxs = []
            for j in range(n):
                nc.tensor.matmul(ch[:, j, :], lhsT=wT[:, 5, :], rhs=hg[j], start=True, stop=True)
            for j in range(n):
                cx = pcx.tile([P, ROWS], F32, tag="cx")
                nc.tensor.matmul(cx, lhsT=wT[:, 2, :], rhs=xg[j], start=True, stop=False)  # stays open
                cxs.append(cx)
            stB[i] = (u_ps, r_ps, cxs, ch)

        u01 = [None]  # shared pair tiles for the two head halves
        t01 = [None]
        uq = {}  # quad tiles for middle pairs, keyed by quad index
        tq = {}

        def stage_c(i):
            u_ps, r_ps, cxs, ch = stB[i]
            gs = ITEMS[i]
            n = len(gs)
            if i <= 1:
                if i == 0:
                    u01[0] = gp.tile([P, 2, ROWS], BF16, tag="u", name="uh01")
                    t01[0] = gp.tile([P, 2, ROWS], BF16, tag="t", name="th01")
                u, t = u01[0][:, i : i + 1, :], t01[0][:, i : i + 1, :]
            elif i == NI - 1:
                # final pair keeps its own tiles for a group-split drain
                u = gp.tile([P, 2, ROWS], BF16, tag="u", name="ulast")
                t = gp.tile([P, 2, ROWS], BF16, tag="t", name="tlast")
                uts[i] = (u.rearrange("p a b -> p (a b)"), t.rearrange("p a b -> p (a b)"))
            else:
                qd, ph = (i - 2) // 2, (i - 2) % 2
                if ph == 0:
                    uq[qd] = gp.tile([P, 2, 2, ROWS], BF16, tag="u", name="uquad")
                    tq[qd] = gp.tile([P, 2, 2, ROWS], BF16, tag="t", name="tquad")
                u, t = uq[qd][:, ph, :, :], tq[qd][:, ph, :, :]
            r = gp.tile([P, 2, ROWS], BF16, tag="r")
            m = gp.tile([P, 2, ROWS], BF16, tag="m")
            nc.scalar.activation(r[:, 0:n, :], r_ps[:, 0:n, :], AF.Sigmoid, bias=bcol[:, 1:2])
            # m per group: shortens the sigR -> m -> id -> tanh_g0 chain so
            # tanh_g0 is ready right as sigU retires (zero ACT bubble)
            for j in range(n):
                nc.vector.scalar_tensor_tensor(
                    m[:, j, :], in0=ch[:, j, :], scalar=bcol[:, 3:4], in1=r[:, j, :],
                    op0=OP.add, op1=OP.mult,
                )
                nc.tensor.matmul(cxs[j], lhsT=ident, rhs=m[:, j, :], start=False, stop=True)
            nc.scalar.activation(u, u_ps[:, 0:n, :], AF.Sigmoid, bias=bcol[:, 0:1])
            for j in range(n):
                nc.scalar.activation(t[:, j, :], cxs[j], AF.Tanh, bias=bcol[:, 2:3])

        def epilogue(uf, tf, base, width):
            hsl = hs[:, base : base + width]
            ua = ep.tile([P, width], BF16, tag="ua", name="ua")
            d = ep.tile([P, width], BF16, tag="d", name="d")
            q = ep.tile([P, width], BF16, tag="q", name="q")
            ho = ep.tile([P, width], BF16, tag="ho", name="ho")
            nc.vector.tensor_tensor(ua, uf, ab[:, base : base + width], OP.mult)
            nc.vector.tensor_tensor(d, tf, hsl, OP.subtract)
            nc.vector.tensor_tensor(q, d, ua, OP.mult)
            nc.vector.tensor_tensor(ho, q, hsl, OP.add)
            nc.sync.dma_start(o_d[:, base : base + width], ho)

        QR = 4 * ROWS  # quad width 2048

        for k in range(NI + 2):
            if k < NI:
                stage_b(k)
            if 1 <= k < NI + 1:
                stage_c(k - 1)
            if k == 3:
                # head halves (groups 0,1) as one 1024-wide epilogue
                epilogue(u01[0].rearrange("p a b -> p (a b)"),
                         t01[0].rearrange("p a b -> p (a b)"), 0, PR)
            if k in (5, 7, 9):
                # quads over item pairs (I2,I3),(I4,I5),(I6,I7)
                qd = (k - 5) // 2
                epilogue(uq[qd].rearrange("p a b c -> p (a b c)"),
                         tq[qd].rearrange("p a b c -> p (a b c)"), PR + qd * QR, QR)
            if k == NI + 1:
                # final pair: per-group chains, first half drains early
                i = NI - 1
                base = ITEMS[i][0] * ROWS
                uf, tf = uts[i]
                for g in range(2):
                    epilogue(uf[:, g * ROWS : (g + 1) * ROWS],
                             tf[:, g * ROWS : (g + 1) * ROWS], base + g * ROWS, ROWS)

    nc.compile()
    return nc


_NC_CACHE = []


def _get_nc():
    if not _NC_CACHE:
        _NC_CACHE.append(build_program())
    return _NC_CACHE[0]


def make_in_maps(x, h_prev, att_score, W_x, b_x, W_h, b_h):
    """Shard + stage inputs for the 8 cores (bf16 wire format)."""
    x = np.asarray(x, dtype=np.float32)
    h_prev = np.asarray(h_prev, dtype=np.float32)
    att = np.asarray(att_score, dtype=np.float32)
    W_x = np.asarray(W_x, dtype=np.float32)
    W_h = np.asarray(W_h, dtype=np.float32)
    b_x = np.asarray(b_x, dtype=np.float32)
    b_h = np.asarray(b_h, dtype=np.float32)

    wxT = np.ascontiguousarray(W_x.T.reshape(I, 3, P).astype(BFNP))
    whT = np.ascontiguousarray(W_h.T.reshape(H, 3, P).astype(BFNP))
    bsum = b_x + b_h  # valid for U and R blocks
    bcol = np.stack(
        [bsum[0:P], bsum[P : 2 * P], b_x[2 * P : 3 * P], b_h[2 * P : 3 * P]], axis=1
    ).astype(np.float32)
    ident = np.eye(P, dtype=BFNP)

    in_maps = []
    for c in range(NCORES):
        s = slice(c * BL, (c + 1) * BL)
        attb = np.broadcast_to(att[s].astype(BFNP), (P, BL))
        in_maps.append(
            {
                "xT": np.ascontiguousarray(x[s].T.astype(BFNP)),
                "hT": np.ascontiguousarray(h_prev[s].T.astype(BFNP)),
                "attb": np.ascontiguousarray(attb),
                "wxT": wxT,
                "whT": whT,
                "bcol": bcol,
                "ident": ident,
            }
        )
    return in_maps


def kernel(x, h_prev, att_score, W_x, b_x, W_h, b_h, **_unused):
    nc = _get_nc()
    in_maps = make_in_maps(x, h_prev, att_score, W_x, b_x, W_h, b_h)
    res = run_bass_kernel_spmd(nc, in_maps, list(range(NCORES)))
    out = np.concatenate(
        [
            np.asarray(res.results[c]["h_newT"]).astype(np.float32).T
            for c in range(NCORES)
        ],
        axis=0,
    )
    return np.ascontiguousarray(out)
